# revision 41
# baseline (speedup 1.0000x reference)
"""Trainium2 Bass kernel for nn_Attention_31705448579931.

Multi-head attention (b=16, L=784, dim=384, H=8, qk=32, v=128) with a
bicubic-resampled relative-position bias:

    out = proj( softmax(q k^T/sqrt(d) + M ab M^T) v )

Sharding: data-parallel over batch - each of the 8 NeuronCores handles 2
batches and all 8 heads.

The bias B = M A M^T depends only on weights, so the host precomputes
U = A M^T per head (A symmetric) and the device adds the bias to each
S^T tile inside the PSUM accumulation group:

    S'^T[k,q] = sum_n M[k,n] U[n,q]  +  sum_d kvec[d,k] qvec[d,q]

The first sum only needs the <=93 M-rows in the 4-tap bicubic window of
each 112-row k-tile, so it is a K=93 matmul whose operands (M windows and
U, both weights-only) live in SBUF for the whole kernel in fp8_e4m3
(bias enters through exp, so ~6% fp8 quantization error on U is ~1e-3
absolute on B - negligible). This removes the bias AllGather of the
earlier design (~140us collective + ~50us PE idle) and all per-instance
bias DMA traffic, and kills the per-tile exp(B) multiply on VectorE.

Device layout highlights:
  - attention is computed k-major: S'^T tiles (kpos on partitions, q on
    free) so exp(S'^T) is directly the lhsT-side operand for the P@V
    matmul; the two N-chunks of each S^T tile run as row-group-packed
    concurrent K=32 matmuls (duplicated q/k rows at partition 32).
  - softmax denominators via an all-ones (112,128) matmul chain on the
    TensorEngine whose output replicates the column sums across
    partitions, followed by a lane-parallel fast reciprocal on VectorE
    and a column-scale fused into the PSUM->SBUF eviction of the
    attention output.
  - PV/ones accumulation chains are flushed one instance behind the
    S'^T/exp stream.
  - q/k projections run just-in-time (one 2-head group ahead); the
    partition-32 row duplication for packing is done by VectorE copies
    (no DMA packets).
  - output is stored bf16 and upcast on the host.

All attention matmuls run in bf16 with fp32 PSUM accumulation; the bias
matmul runs in fp8_e4m3 (U pre-scaled by 8, M windows by 1/8).
"""

import numpy as np
import ml_dtypes

import concourse.mybir as mybir
import concourse.tile as tile
from concourse import bacc
from concourse.bass_utils import run_bass_kernel_spmd

N_CORES = 8
B = 16          # global batch
BC = B // N_CORES  # batches per core
L = 784
DIM = 384
H = 8
QK = 32
VD = 128
RES = 25
N = RES * RES   # 625
A_CUBIC = -0.75
SCALE = QK ** -0.5

LT = 7          # l tiles of 112
LTS = 112
WMAX = 93       # max 4-tap window rows per l-tile (bias contraction)
USCALE = 8.0    # fp8 pre-scale: u *= USCALE, mtw /= USCALE
F32 = mybir.dt.float32
BF16 = mybir.dt.bfloat16
FP8 = mybir.dt.float8e4
BF16_NP = ml_dtypes.bfloat16
FP8_NP = ml_dtypes.float8_e4m3fn

NSPLITS = [(0, 512), (512, L)]  # free-dim chunks for 784-wide matmul outputs


def _cubic_weight(x):
    ax = np.abs(x)
    a = A_CUBIC
    w1 = ((a + 2.0) * ax - (a + 3.0)) * ax * ax + 1.0
    w2 = a * (((ax - 5.0) * ax + 8.0) * ax - 4.0)
    return np.where(ax <= 1.0, w1, np.where(ax < 2.0, w2, 0.0)).astype(np.float32)


def interp_matrix(Lo, Li):
    """Dense 1-D bicubic resampling matrix (Lo, Li), matches reference."""
    scale = Li / Lo
    src = (np.arange(Lo, dtype=np.float32) + 0.5) * scale - 0.5
    f = np.floor(src)
    t = (src - f).astype(np.float32)
    ws = np.stack(
        [_cubic_weight(t + 1.0), _cubic_weight(t), _cubic_weight(1.0 - t),
         _cubic_weight(2.0 - t)], axis=1)
    idx = f.astype(np.int32)[:, None] + np.arange(-1, 3, dtype=np.int32)[None, :]
    idx = np.clip(idx, 0, Li - 1)
    M = np.zeros((Lo, Li), dtype=np.float32)
    np.add.at(M, (np.arange(Lo)[:, None], idx), ws)
    return M


def _tap_windows():
    """Per l-tile row window [lo, hi) of M columns feeding that tile (4-tap)."""
    scale = N / L
    src = (np.arange(L, dtype=np.float64) + 0.5) * scale - 0.5
    f = np.floor(src).astype(np.int64)
    lo_tap = np.clip(f - 1, 0, N - 1)
    hi_tap = np.clip(f + 2, 0, N - 1)
    wins = []
    for pt in range(LT):
        sl = slice(pt * LTS, (pt + 1) * LTS)
        wins.append((int(lo_tap[sl].min()), int(hi_tap[sl].max()) + 1))
    return wins


_BUILD_CACHE = {}


def build():
    if "nc" in _BUILD_CACHE:
        return _BUILD_CACHE["nc"]

    nc = bacc.Bacc("TRN2", target_bir_lowering=False, debug=False,
                   num_devices=N_CORES)

    xT_e = nc.dram_tensor("xT", [BC, DIM, L], BF16, kind="ExternalInput")
    wqkvT_e = nc.dram_tensor("wqkvT", [DIM, 1536], BF16, kind="ExternalInput")
    wprojT_e = nc.dram_tensor("wprojT", [H * VD, DIM], BF16, kind="ExternalInput")
    bproj_e = nc.dram_tensor("bproj", [1, DIM], F32, kind="ExternalInput")
    # bias operands (weights-only, fp8): mtw8[r, kt*112+c] = M[kt*112+c,
    # lo_kt+r]/USCALE, u8[r, (h*7+kt)*784+q] = USCALE*U_h[lo_kt+r, q],
    # both 0-padded to 93 rows.
    mtw_e = nc.dram_tensor("mtw8", [WMAX, L], FP8, kind="ExternalInput")
    u_e = nc.dram_tensor("u8", [WMAX, H * LT * L], FP8, kind="ExternalInput")
    out_e = nc.dram_tensor("out", [BC, L, DIM], BF16, kind="ExternalOutput")

    with tile.TileContext(nc) as tc:
        with (
            tc.tile_pool(name="const", bufs=1) as constp,
            tc.tile_pool(name="wq", bufs=1) as wqp,
            tc.tile_pool(name="x", bufs=1) as xp,
            tc.tile_pool(name="v", bufs=1) as vp,
            tc.tile_pool(name="qk", bufs=1) as qkp,
            tc.tile_pool(name="attn", bufs=3) as attnp,
            tc.tile_pool(name="mis", bufs=1) as misp,
            tc.tile_pool(name="ot", bufs=1) as otp,
            tc.tile_pool(name="ps", bufs=4, space="PSUM") as psp,
        ):
            # ---- inputs ---------------------------------------------------
            # x first (gates V/qk projections), weights next, then the
            # resident fp8 bias operands spread across the three queues.
            xT_sb = [[None] * 3 for _ in range(BC)]
            for b in range(BC):
                for kc in range(3):
                    t = xp.tile([128, L], BF16, tag=f"x{b}{kc}")
                    (nc.sync if b == 0 else nc.gpsimd).dma_start(
                        t[:], xT_e[b, kc * 128:(kc + 1) * 128, :])
                    xT_sb[b][kc] = t
            wq_sb = []
            for kc in range(3):
                t = wqp.tile([128, 1536], BF16, tag=f"wq{kc}")
                nc.scalar.dma_start(t[:], wqkvT_e[kc * 128:(kc + 1) * 128, :])
                wq_sb.append(t)

            mtw_sb = constp.tile([WMAX, L], FP8, tag="mtw")
            nc.scalar.dma_start(mtw_sb[:], mtw_e[:, :])
            u_sb = constp.tile([WMAX, H * LT * L], FP8, tag="u8")
            for h in range(H):
                eng = (nc.gpsimd, nc.sync, nc.scalar)[h % 3]
                eng.dma_start(u_sb[:, h * LT * L:(h + 1) * LT * L],
                              u_e[:, h * LT * L:(h + 1) * LT * L])

            bproj_row = constp.tile([1, DIM], F32, tag="bprow")
            nc.gpsimd.dma_start(bproj_row[:], bproj_e[:, :])
            bpb = constp.tile([128, DIM], F32, tag="bpb")
            nc.gpsimd.partition_broadcast(bpb[:], bproj_row[:])
            ones_l = constp.tile([LTS, VD], BF16, tag="ones")
            nc.any.memset(ones_l[:], 1.0)

            wproj_sb = []
            for h in range(H):
                t = wqp.tile([128, DIM], BF16, tag=f"wp{h}")
                nc.gpsimd.dma_start(t[:], wprojT_e[h * VD:(h + 1) * VD, :])
                wproj_sb.append(t)

            # ---- PE warmup: dummy matmuls on constants run during the
            # input-DMA wait and release the HAM clock throttle
            # (1.2 -> 2.4 GHz) before the first real matmul burst.
            warm_rhs = constp.tile([LTS, 512], BF16, tag="warmr")
            nc.any.memset(warm_rhs[:], 0.5)
            warm_ps = psp.tile([VD, 512], F32, tag="ps")
            for wi in range(24):
                nc.tensor.matmul(
                    warm_ps[:],
                    lhsT=ones_l[:],
                    rhs=warm_rhs[:],
                    start=(wi == 0), stop=(wi == 23),
                )

            # ---- V projection (l-major: kpos on partitions) ---------------
            v_sb = [[None] * LT for _ in range(BC)]

            def _vproj(b):
                for lt in range(LT):
                    pv = psp.tile([LTS, 1024], F32, tag="ps")
                    for half in range(2):
                        for kc in range(3):
                            nc.tensor.matmul(
                                pv[:, half * 512:(half + 1) * 512],
                                lhsT=xT_sb[b][kc][:, lt * LTS:(lt + 1) * LTS],
                                rhs=wq_sb[kc][:, 512 + half * 512:1024 + half * 512],
                                start=(kc == 0), stop=(kc == 2),
                            )
                    vt = vp.tile([LTS, 1024], BF16, tag=f"v{b}{lt}")
                    if lt % 2 == 0:
                        nc.scalar.copy(vt[:], pv[:])
                    else:
                        nc.vector.tensor_copy(vt[:], pv[:])
                    v_sb[b][lt] = vt

            _vproj(0)

            # ---- per-head loop --------------------------------------------
            qk_sb = {}
            ot_sb = [[None] * H for _ in range(BC)]
            _pend = []

            def _qkproj(mt_i):
                """q/k projections for heads (2*mt_i, 2*mt_i+1), both
                batches, with the rows duplicated to partitions 32:64 by
                VectorE for 2-way row-group packing of the S matmuls."""
                for b in range(BC):
                    pqk = psp.tile([128, L], F32, tag="ps")
                    for (ns, ne) in NSPLITS:
                        for kc in range(3):
                            nc.tensor.matmul(
                                pqk[:, ns:ne],
                                lhsT=wq_sb[kc][:, mt_i * 128:(mt_i + 1) * 128],
                                rhs=xT_sb[b][kc][:, ns:ne],
                                start=(kc == 0), stop=(kc == 2),
                            )
                    for hh in range(2):
                        h_ = 2 * mt_i + hh
                        inst = 2 * h_ + b
                        qt = qkp.tile([2 * QK, L], BF16, tag=f"q{inst % 4}")
                        kt_ = qkp.tile([2 * QK, L], BF16, tag=f"k{inst % 4}")
                        nc.scalar.copy(qt[0:QK, :], pqk[hh * 64:hh * 64 + 32, :])
                        nc.vector.tensor_copy(kt_[0:QK, :], pqk[hh * 64 + 32:hh * 64 + 64, :])
                        nc.vector.tensor_copy(qt[QK:2 * QK, :], qt[0:QK, :])
                        nc.vector.tensor_copy(kt_[QK:2 * QK, :], kt_[0:QK, :])
                        qk_sb[(b, h_)] = (qt, kt_)

            def _flush_pv_gen(item):
                """Generator: emits the ones/PV chain matmuls in small
                steps so the caller can interleave them between the next
                instance's S' tiles - the chain matmuls keep the PE busy
                while ScalarE works through the exps, instead of the two
                phases alternating with ScalarE idle half the time."""
                fh, fb, ptiles = item
                ps_o = psp.tile([VD, L], F32, tag="ps")
                ps_one = psp.tile([VD, L], F32, tag="ps")
                steps = []
                for (ns, ne) in NSPLITS:
                    for kt in range(LT):
                        steps.append(("one", kt, ns, ne))
                for (ns, ne) in NSPLITS:
                    for kt in range(LT):
                        steps.append(("pv", kt, ns, ne))
                rdb = None
                for si, (kind, kt, ns, ne) in enumerate(steps):
                    if kind == "one":
                        nc.tensor.matmul(
                            ps_one[:, ns:ne],
                            lhsT=ones_l[:],
                            rhs=ptiles[kt][:, ns:ne],
                            start=(kt == 0), stop=(kt == LT - 1),
                        )
                        if si == LT * 2 - 1:
                            rdb = misp.tile([VD, L], F32, tag="rdb")
                            nc.vector.reciprocal_approx_fast(rdb[:], ps_one[:])
                    else:
                        nc.tensor.matmul(
                            ps_o[:, ns:ne],
                            lhsT=v_sb[fb][kt][:, fh * VD:(fh + 1) * VD],
                            rhs=ptiles[kt][:, ns:ne],
                            start=(kt == 0), stop=(kt == LT - 1),
                        )
                    yield
                ot = otp.tile([VD, L], BF16, tag=f"ot{fb}{fh}")
                nc.vector.tensor_mul(ot[:], ps_o[:], rdb[:])
                ot_sb[fb][fh] = ot
                while True:
                    yield

            def _flush_pv(item):
                g = _flush_pv_gen(item)
                for _ in range(LT * 4 + 1):
                    next(g)

            def _proj(b):
                for lt in range(LT):
                    py = psp.tile([LTS, DIM], F32, tag="ps")
                    for h in range(H):
                        nc.tensor.matmul(
                            py[:],
                            lhsT=ot_sb[b][h][:, lt * LTS:(lt + 1) * LTS],
                            rhs=wproj_sb[h][:],
                            start=(h == 0), stop=(h == H - 1),
                        )
                    # distinct ysb buffer per tile: with a single shared
                    # buffer each add had to wait for the previous store
                    # DMA to finish, serializing the whole drain at
                    # ~2.9us/tile and blocking the last flush's DVE ops.
                    ysb = misp.tile([LTS, DIM], BF16, tag=f"y{b}{lt}")
                    nc.vector.tensor_add(ysb[:], py[:], bpb[:LTS, :])
                    eng = (nc.sync, nc.scalar, nc.gpsimd)[lt % 3]
                    eng.dma_start(out_e[b, lt * LTS:(lt + 1) * LTS, :], ysb[:])

            # qkproj(0) between the two V batches: emitted after all 14 V
            # tiles, its pqk allocation stalls the PE ~5.6us waiting for
            # the V-copy backlog to free a psum buffer (and trips the HAM
            # throttle); after 7 tiles the pool has already drained.
            _qkproj(0)
            _vproj(1)
            for h in range(H):
                for b in range(BC):
                    # prefetch the next 2-head group's q/k projections one
                    # head ahead of first use so the CASTs overlap the
                    # current instances instead of stalling the next ones.
                    if len(_pend) > 1:
                        _flush_pv(_pend.pop(0))
                    # prefetch after the flush: the chain matmuls keep the
                    # PE busy while exps drain the psum pool, so the pqk
                    # allocation no longer stalls the engine (observed
                    # 4.4us gap -> HAM re-throttle cascade when emitted
                    # before the flush).
                    if h % 2 == 1 and b == 0 and h < H - 1:
                        _qkproj((h + 1) // 2)
                    qt, kt_ = qk_sb[(b, h)]
                    pt_tiles = []
                    for kt in range(LT):
                        ps_s = psp.tile([LTS, L], F32, tag="ps")
                        # bias: B^T tile = mtw8^T @ u8 window (fp8, K=93)
                        for (ns, ne) in NSPLITS:
                            nc.tensor.matmul(
                                ps_s[:, ns:ne],
                                lhsT=mtw_sb[:, kt * LTS:(kt + 1) * LTS],
                                rhs=u_sb[:, (h * LT + kt) * L + ns:(h * LT + kt) * L + ne],
                                start=True, stop=False,
                            )
                        # S: the two N-chunks run as row-group-packed
                        # concurrent K=32 matmuls accumulating onto B^T.
                        for wi, (ns, ne) in enumerate(NSPLITS):
                            o0 = wi * QK
                            nc.tensor.matmul(
                                ps_s[:, ns:ne],
                                lhsT=kt_[o0:o0 + QK, kt * LTS:(kt + 1) * LTS],
                                rhs=qt[o0:o0 + QK, ns:ne],
                                start=False, stop=True,
                            )
                        ptile = attnp.tile([LTS, L], BF16, tag=f"pT{kt}")
                        nc.scalar.activation(ptile[:], ps_s[:],
                                             mybir.ActivationFunctionType.Exp)
                        pt_tiles.append(ptile)
                    _pend.append((h, b, pt_tiles))

            # ---- drain: flush both remaining instances first (their
            # chains have no unmet deps), then both projections - the
            # projections' ot operands are DVE-produced, so putting the
            # chains first keeps the PE busy while DVE catches up.
            while _pend:
                _flush_pv(_pend.pop(0))
            for b in range(BC):
                _proj(b)

    nc.compile()
    _BUILD_CACHE["nc"] = nc
    return nc


def _prep_in_maps(inputs):
    x = np.asarray(inputs["x"], dtype=np.float32)
    Wqkv = np.asarray(inputs["Wqkv"], dtype=np.float32)
    Wproj = np.asarray(inputs["Wproj"], dtype=np.float32)
    bproj = np.asarray(inputs["bproj"], dtype=np.float32)
    ab_table = np.asarray(inputs["ab_table"], dtype=np.float32)
    bias_idxs = np.asarray(inputs["bias_idxs"])

    # reorder qkv weight rows: [q0 k0 q1 k1 ... q7 k7 | v0 v1 ... v7]
    w3 = Wqkv.reshape(H, 2 * QK + VD, DIM)
    qk_rows = np.concatenate(
        [np.concatenate([w3[h, :QK] * SCALE, w3[h, QK:2 * QK]], axis=0)
         for h in range(H)],
        axis=0)                     # (512, 384)
    v_rows = np.concatenate([w3[h, 2 * QK:] for h in range(H)], axis=0)  # (1024,384)
    wqkvT = np.ascontiguousarray(
        np.concatenate([qk_rows, v_rows], axis=0).T).astype(BF16_NP)  # (384,1536)

    wprojT = np.ascontiguousarray(Wproj.T).astype(BF16_NP)  # (1024, 384)
    bproj2 = np.ascontiguousarray(bproj.reshape(1, DIM))

    # bias fold: B_h = M A_h M^T with A_h symmetric; device computes
    # B^T[k,q] = sum_{n in win} M[k,n] U_h[n,q] as a K=93 fp8 matmul.
    M = interp_matrix(L, N)                      # (784, 625)
    A = ab_table[:, bias_idxs]                   # (8, 625, 625)
    U = np.einsum('hnm,qm->hnq', A, M, optimize=True)  # (8, 625, 784)
    wins = _tap_windows()
    u_pack = np.zeros((WMAX, H * LT * L), np.float32)
    mtw = np.zeros((WMAX, L), np.float32)
    for kt, (lo, hi) in enumerate(wins):
        w = hi - lo
        for h in range(H):
            u_pack[:w, (h * LT + kt) * L:(h * LT + kt + 1) * L] = U[h, lo:hi, :]
        mtw[:w, kt * LTS:(kt + 1) * LTS] = M[kt * LTS:(kt + 1) * LTS, lo:hi].T
    u8 = np.ascontiguousarray(u_pack * USCALE).astype(FP8_NP)
    mtw8 = np.ascontiguousarray(mtw / USCALE).astype(FP8_NP)

    in_maps = []
    for c in range(N_CORES):
        xT = np.ascontiguousarray(
            x[c * BC:(c + 1) * BC].transpose(0, 2, 1)).astype(BF16_NP)
        in_maps.append({
            "xT": xT,
            "wqkvT": wqkvT,
            "wprojT": wprojT,
            "bproj": bproj2,
            "mtw8": mtw8,
            "u8": u8,
        })
    return in_maps


def _run(inputs, trace=False, **kw):
    nc = build()
    in_maps = _prep_in_maps(inputs)
    last_err = None
    for attempt in range(3):
        try:
            res = run_bass_kernel_spmd(nc, in_maps, core_ids=list(range(N_CORES)),
                                       trace=trace, **kw)
            break
        except Exception as e:  # transient NRT device errors: retry
            last_err = e
    else:
        raise last_err
    out = np.concatenate(
        [np.asarray(res.results[c]["out"], dtype=np.float32)
         for c in range(N_CORES)], axis=0)
    return out, res


def kernel(**inputs) -> np.ndarray:
    out, _ = _run(inputs, trace=False)
    return out


# revision 43
# speedup vs baseline: 1.1389x; 1.1389x over previous
"""Trainium2 Bass kernel for nn_Attention_31705448579931.

Multi-head attention (b=16, L=784, dim=384, H=8, qk=32, v=128) with a
bicubic-resampled relative-position bias:

    out = proj( softmax(q k^T/sqrt(d) + M ab M^T) v )

Sharding: data-parallel over batch - each of the 8 NeuronCores handles 2
batches and all 8 heads.

The bias B = M A M^T depends only on weights, so the host precomputes
U = A M^T per head (A symmetric) and the device adds the bias to each
S^T tile inside the PSUM accumulation group:

    S'^T[k,q] = sum_n M[k,n] U[n,q]  +  sum_d kvec[d,k] qvec[d,q]

The first sum only needs the <=93 M-rows in the 4-tap bicubic window of
each 112-row k-tile, so it is a K=93 matmul whose operands (M windows and
U, both weights-only) live in SBUF for the whole kernel in fp8_e4m3
(bias enters through exp, so ~6% fp8 quantization error on U is ~1e-3
absolute on B - negligible). This removes the bias AllGather of the
earlier design (~140us collective + ~50us PE idle) and all per-instance
bias DMA traffic, and kills the per-tile exp(B) multiply on VectorE.

Device layout highlights:
  - attention is computed k-major: S'^T tiles (kpos on partitions, q on
    free) so exp(S'^T) is directly the lhsT-side operand for the P@V
    matmul; the two N-chunks of each S^T tile run as row-group-packed
    concurrent K=32 matmuls (duplicated q/k rows at partition 32).
  - softmax denominators via an all-ones (112,128) matmul chain on the
    TensorEngine whose output replicates the column sums across
    partitions, followed by a lane-parallel fast reciprocal on VectorE
    and a column-scale fused into the PSUM->SBUF eviction of the
    attention output.
  - PV/ones accumulation chains are flushed one instance behind the
    S'^T/exp stream.
  - q/k projections run just-in-time (one 2-head group ahead); the
    partition-32 row duplication for packing is done by VectorE copies
    (no DMA packets).
  - output is stored bf16 and upcast on the host.

All attention matmuls run in bf16 with fp32 PSUM accumulation; the bias
matmul runs in fp8_e4m3 (U pre-scaled by 8, M windows by 1/8).
"""

import numpy as np
import ml_dtypes

import concourse.mybir as mybir
import concourse.tile as tile
from concourse import bacc
from concourse.bass_utils import run_bass_kernel_spmd

N_CORES = 8
B = 16          # global batch
BC = B // N_CORES  # batches per core
L = 784
DIM = 384
H = 8
QK = 32
VD = 128
RES = 25
N = RES * RES   # 625
A_CUBIC = -0.75
SCALE = QK ** -0.5

LT = 7          # l tiles of 112
LTS = 112
WMAX = 93       # max 4-tap window rows per l-tile (bias contraction)
USCALE = 8.0    # fp8 pre-scale: u *= USCALE, mtw /= USCALE
F32 = mybir.dt.float32
BF16 = mybir.dt.bfloat16
FP8 = mybir.dt.float8e4
BF16_NP = ml_dtypes.bfloat16
FP8_NP = ml_dtypes.float8_e4m3fn

NSPLITS = [(0, 512), (512, L)]  # free-dim chunks for 784-wide matmul outputs


def _cubic_weight(x):
    ax = np.abs(x)
    a = A_CUBIC
    w1 = ((a + 2.0) * ax - (a + 3.0)) * ax * ax + 1.0
    w2 = a * (((ax - 5.0) * ax + 8.0) * ax - 4.0)
    return np.where(ax <= 1.0, w1, np.where(ax < 2.0, w2, 0.0)).astype(np.float32)


def interp_matrix(Lo, Li):
    """Dense 1-D bicubic resampling matrix (Lo, Li), matches reference."""
    scale = Li / Lo
    src = (np.arange(Lo, dtype=np.float32) + 0.5) * scale - 0.5
    f = np.floor(src)
    t = (src - f).astype(np.float32)
    ws = np.stack(
        [_cubic_weight(t + 1.0), _cubic_weight(t), _cubic_weight(1.0 - t),
         _cubic_weight(2.0 - t)], axis=1)
    idx = f.astype(np.int32)[:, None] + np.arange(-1, 3, dtype=np.int32)[None, :]
    idx = np.clip(idx, 0, Li - 1)
    M = np.zeros((Lo, Li), dtype=np.float32)
    np.add.at(M, (np.arange(Lo)[:, None], idx), ws)
    return M


def _tap_windows():
    """Per l-tile row window [lo, hi) of M columns feeding that tile (4-tap)."""
    scale = N / L
    src = (np.arange(L, dtype=np.float64) + 0.5) * scale - 0.5
    f = np.floor(src).astype(np.int64)
    lo_tap = np.clip(f - 1, 0, N - 1)
    hi_tap = np.clip(f + 2, 0, N - 1)
    wins = []
    for pt in range(LT):
        sl = slice(pt * LTS, (pt + 1) * LTS)
        wins.append((int(lo_tap[sl].min()), int(hi_tap[sl].max()) + 1))
    return wins


_BUILD_CACHE = {}


def build():
    if "nc" in _BUILD_CACHE:
        return _BUILD_CACHE["nc"]

    nc = bacc.Bacc("TRN2", target_bir_lowering=False, debug=False,
                   num_devices=N_CORES)

    xT_e = nc.dram_tensor("xT", [BC, DIM, L], BF16, kind="ExternalInput")
    wqkvT_e = nc.dram_tensor("wqkvT", [DIM, 1536], BF16, kind="ExternalInput")
    wprojT_e = nc.dram_tensor("wprojT", [H * VD, DIM], BF16, kind="ExternalInput")
    bproj_e = nc.dram_tensor("bproj", [1, DIM], F32, kind="ExternalInput")
    # bias operands (weights-only, fp8): mtw8[r, kt*112+c] = M[kt*112+c,
    # lo_kt+r]/USCALE, u8[r, (h*7+kt)*784+q] = USCALE*U_h[lo_kt+r, q],
    # both 0-padded to 93 rows.
    mtw_e = nc.dram_tensor("mtw8", [WMAX, L], FP8, kind="ExternalInput")
    u_e = nc.dram_tensor("u8", [WMAX, H * LT * L], FP8, kind="ExternalInput")
    out_e = nc.dram_tensor("out", [BC, L, DIM], BF16, kind="ExternalOutput")

    with tile.TileContext(nc) as tc:
        with (
            tc.tile_pool(name="const", bufs=1) as constp,
            tc.tile_pool(name="wq", bufs=1) as wqp,
            tc.tile_pool(name="x", bufs=1) as xp,
            tc.tile_pool(name="v", bufs=1) as vp,
            tc.tile_pool(name="qk", bufs=1) as qkp,
            tc.tile_pool(name="attn", bufs=3) as attnp,
            tc.tile_pool(name="mis", bufs=1) as misp,
            tc.tile_pool(name="ot", bufs=1) as otp,
            tc.tile_pool(name="ps", bufs=4, space="PSUM") as psp,
        ):
            # ---- inputs ---------------------------------------------------
            # x first (gates V/qk projections), weights next, then the
            # resident fp8 bias operands spread across the three queues.
            xT_sb = [[None] * 3 for _ in range(BC)]
            for b in range(BC):
                for kc in range(3):
                    t = xp.tile([128, L], BF16, tag=f"x{b}{kc}")
                    (nc.sync if b == 0 else nc.gpsimd).dma_start(
                        t[:], xT_e[b, kc * 128:(kc + 1) * 128, :])
                    xT_sb[b][kc] = t
            wq_sb = []
            for kc in range(3):
                t = wqp.tile([128, 1536], BF16, tag=f"wq{kc}")
                nc.scalar.dma_start(t[:], wqkvT_e[kc * 128:(kc + 1) * 128, :])
                wq_sb.append(t)

            mtw_sb = constp.tile([WMAX, L], FP8, tag="mtw")
            nc.scalar.dma_start(mtw_sb[:], mtw_e[:, :])
            u_sb = constp.tile([WMAX, H * LT * L], FP8, tag="u8")
            for h in range(H):
                eng = (nc.gpsimd, nc.sync, nc.scalar)[h % 3]
                eng.dma_start(u_sb[:, h * LT * L:(h + 1) * LT * L],
                              u_e[:, h * LT * L:(h + 1) * LT * L])

            bproj_row = constp.tile([1, DIM], F32, tag="bprow")
            nc.gpsimd.dma_start(bproj_row[:], bproj_e[:, :])
            bpb = constp.tile([128, DIM], F32, tag="bpb")
            nc.gpsimd.partition_broadcast(bpb[:], bproj_row[:])
            ones_l = constp.tile([LTS, VD], BF16, tag="ones")
            nc.any.memset(ones_l[:], 1.0)

            wproj_sb = []
            for h in range(H):
                t = wqp.tile([128, DIM], BF16, tag=f"wp{h}")
                nc.gpsimd.dma_start(t[:], wprojT_e[h * VD:(h + 1) * VD, :])
                wproj_sb.append(t)

            # ---- PE warmup: dummy matmuls on constants run during the
            # input-DMA wait and release the HAM clock throttle
            # (1.2 -> 2.4 GHz) before the first real matmul burst.
            warm_rhs = constp.tile([LTS, 512], BF16, tag="warmr")
            nc.any.memset(warm_rhs[:], 0.5)
            warm_ps = psp.tile([VD, 512], F32, tag="ps")
            for wi in range(24):
                nc.tensor.matmul(
                    warm_ps[:],
                    lhsT=ones_l[:],
                    rhs=warm_rhs[:],
                    start=(wi == 0), stop=(wi == 23),
                )

            # ---- V projection (l-major: kpos on partitions) ---------------
            v_sb = [[None] * LT for _ in range(BC)]

            def _vproj(b):
                for lt in range(LT):
                    pv = psp.tile([LTS, 1024], F32, tag="ps")
                    for half in range(2):
                        for kc in range(3):
                            nc.tensor.matmul(
                                pv[:, half * 512:(half + 1) * 512],
                                lhsT=xT_sb[b][kc][:, lt * LTS:(lt + 1) * LTS],
                                rhs=wq_sb[kc][:, 512 + half * 512:1024 + half * 512],
                                start=(kc == 0), stop=(kc == 2),
                            )
                    vt = vp.tile([LTS, 1024], BF16, tag=f"v{b}{lt}")
                    if lt % 2 == 0:
                        nc.scalar.copy(vt[:], pv[:])
                    else:
                        nc.vector.tensor_copy(vt[:], pv[:])
                    v_sb[b][lt] = vt

            _vproj(0)
            _vproj(1)

            # ---- per-head loop --------------------------------------------
            qk_sb = {}
            ot_sb = [[None] * H for _ in range(BC)]
            _pend = []

            def _qkproj(mt_i):
                """q/k projections for heads (2*mt_i, 2*mt_i+1), both
                batches, with the rows duplicated to partitions 32:64 by
                VectorE for 2-way row-group packing of the S matmuls."""
                for b in range(BC):
                    pqk = psp.tile([128, L], F32, tag="ps")
                    for (ns, ne) in NSPLITS:
                        for kc in range(3):
                            nc.tensor.matmul(
                                pqk[:, ns:ne],
                                lhsT=wq_sb[kc][:, mt_i * 128:(mt_i + 1) * 128],
                                rhs=xT_sb[b][kc][:, ns:ne],
                                start=(kc == 0), stop=(kc == 2),
                            )
                    for hh in range(2):
                        h_ = 2 * mt_i + hh
                        inst = 2 * h_ + b
                        qt = qkp.tile([2 * QK, L], BF16, tag=f"q{inst % 4}")
                        kt_ = qkp.tile([2 * QK, L], BF16, tag=f"k{inst % 4}")
                        nc.scalar.copy(qt[0:QK, :], pqk[hh * 64:hh * 64 + 32, :])
                        nc.vector.tensor_copy(kt_[0:QK, :], pqk[hh * 64 + 32:hh * 64 + 64, :])
                        nc.vector.tensor_copy(qt[QK:2 * QK, :], qt[0:QK, :])
                        nc.vector.tensor_copy(kt_[QK:2 * QK, :], kt_[0:QK, :])
                        qk_sb[(b, h_)] = (qt, kt_)

            def _flush_pv_gen(item):
                """Generator: emits the ones/PV chain matmuls in small
                steps so the caller can interleave them between the next
                instance's S' tiles - the chain matmuls keep the PE busy
                while ScalarE works through the exps, instead of the two
                phases alternating with ScalarE idle half the time."""
                fh, fb, ptiles = item
                ps_o = psp.tile([VD, L], F32, tag="ps")
                ps_one = psp.tile([VD, L], F32, tag="ps")
                steps = []
                for (ns, ne) in NSPLITS:
                    for kt in range(LT):
                        steps.append(("one", kt, ns, ne))
                for (ns, ne) in NSPLITS:
                    for kt in range(LT):
                        steps.append(("pv", kt, ns, ne))
                rdb = None
                for si, (kind, kt, ns, ne) in enumerate(steps):
                    if kind == "one":
                        nc.tensor.matmul(
                            ps_one[:, ns:ne],
                            lhsT=ones_l[:],
                            rhs=ptiles[kt][:, ns:ne],
                            start=(kt == 0), stop=(kt == LT - 1),
                        )
                        if si == LT * 2 - 1:
                            rdb = misp.tile([VD, L], F32, tag="rdb")
                            nc.vector.reciprocal_approx_fast(rdb[:], ps_one[:])
                    else:
                        nc.tensor.matmul(
                            ps_o[:, ns:ne],
                            lhsT=v_sb[fb][kt][:, fh * VD:(fh + 1) * VD],
                            rhs=ptiles[kt][:, ns:ne],
                            start=(kt == 0), stop=(kt == LT - 1),
                        )
                    yield
                ot = otp.tile([VD, L], BF16, tag=f"ot{fb}{fh}")
                nc.vector.tensor_mul(ot[:], ps_o[:], rdb[:])
                ot_sb[fb][fh] = ot
                while True:
                    yield

            def _flush_pv(item):
                g = _flush_pv_gen(item)
                for _ in range(LT * 4 + 1):
                    next(g)

            def _proj(b):
                for lt in range(LT):
                    py = psp.tile([LTS, DIM], F32, tag="ps")
                    for h in range(H):
                        nc.tensor.matmul(
                            py[:],
                            lhsT=ot_sb[b][h][:, lt * LTS:(lt + 1) * LTS],
                            rhs=wproj_sb[h][:],
                            start=(h == 0), stop=(h == H - 1),
                        )
                    # distinct ysb buffer per tile: with a single shared
                    # buffer each add had to wait for the previous store
                    # DMA to finish, serializing the whole drain at
                    # ~2.9us/tile and blocking the last flush's DVE ops.
                    ysb = misp.tile([LTS, DIM], BF16, tag=f"y{b}{lt}")
                    nc.vector.tensor_add(ysb[:], py[:], bpb[:LTS, :])
                    eng = (nc.sync, nc.scalar, nc.gpsimd)[lt % 3]
                    eng.dma_start(out_e[b, lt * LTS:(lt + 1) * LTS, :], ysb[:])

            _qkproj(0)
            for h in range(H):
                for b in range(BC):
                    # prefetch the next 2-head group's q/k projections one
                    # head ahead of first use so the CASTs overlap the
                    # current instances instead of stalling the next ones.
                    if len(_pend) > 1:
                        _flush_pv(_pend.pop(0))
                    # prefetch after the flush: the chain matmuls keep the
                    # PE busy while exps drain the psum pool, so the pqk
                    # allocation no longer stalls the engine (observed
                    # 4.4us gap -> HAM re-throttle cascade when emitted
                    # before the flush).
                    if h % 2 == 1 and b == 0 and h < H - 1:
                        _qkproj((h + 1) // 2)
                    qt, kt_ = qk_sb[(b, h)]
                    pt_tiles = []
                    for kt in range(LT):
                        ps_s = psp.tile([LTS, L], F32, tag="ps")
                        # bias: B^T tile = mtw8^T @ u8 window (fp8, K=93)
                        for (ns, ne) in NSPLITS:
                            nc.tensor.matmul(
                                ps_s[:, ns:ne],
                                lhsT=mtw_sb[:, kt * LTS:(kt + 1) * LTS],
                                rhs=u_sb[:, (h * LT + kt) * L + ns:(h * LT + kt) * L + ne],
                                start=True, stop=False,
                            )
                        # S: the two N-chunks run as row-group-packed
                        # concurrent K=32 matmuls accumulating onto B^T.
                        for wi, (ns, ne) in enumerate(NSPLITS):
                            o0 = wi * QK
                            nc.tensor.matmul(
                                ps_s[:, ns:ne],
                                lhsT=kt_[o0:o0 + QK, kt * LTS:(kt + 1) * LTS],
                                rhs=qt[o0:o0 + QK, ns:ne],
                                start=False, stop=True,
                            )
                        ptile = attnp.tile([LTS, L], BF16, tag=f"pT{kt}")
                        nc.scalar.activation(ptile[:], ps_s[:],
                                             mybir.ActivationFunctionType.Exp)
                        pt_tiles.append(ptile)
                    _pend.append((h, b, pt_tiles))

            # ---- drain: flush both remaining instances first (their
            # chains have no unmet deps), then both projections - the
            # projections' ot operands are DVE-produced, so putting the
            # chains first keeps the PE busy while DVE catches up.
            while _pend:
                _flush_pv(_pend.pop(0))
            for b in range(BC):
                _proj(b)

    nc.compile()
    _BUILD_CACHE["nc"] = nc
    return nc


def _prep_in_maps(inputs):
    x = np.asarray(inputs["x"], dtype=np.float32)
    Wqkv = np.asarray(inputs["Wqkv"], dtype=np.float32)
    Wproj = np.asarray(inputs["Wproj"], dtype=np.float32)
    bproj = np.asarray(inputs["bproj"], dtype=np.float32)
    ab_table = np.asarray(inputs["ab_table"], dtype=np.float32)
    bias_idxs = np.asarray(inputs["bias_idxs"])

    # reorder qkv weight rows: [q0 k0 q1 k1 ... q7 k7 | v0 v1 ... v7]
    w3 = Wqkv.reshape(H, 2 * QK + VD, DIM)
    qk_rows = np.concatenate(
        [np.concatenate([w3[h, :QK] * SCALE, w3[h, QK:2 * QK]], axis=0)
         for h in range(H)],
        axis=0)                     # (512, 384)
    v_rows = np.concatenate([w3[h, 2 * QK:] for h in range(H)], axis=0)  # (1024,384)
    wqkvT = np.ascontiguousarray(
        np.concatenate([qk_rows, v_rows], axis=0).T).astype(BF16_NP)  # (384,1536)

    wprojT = np.ascontiguousarray(Wproj.T).astype(BF16_NP)  # (1024, 384)
    bproj2 = np.ascontiguousarray(bproj.reshape(1, DIM))

    # bias fold: B_h = M A_h M^T with A_h symmetric; device computes
    # B^T[k,q] = sum_{n in win} M[k,n] U_h[n,q] as a K=93 fp8 matmul.
    M = interp_matrix(L, N)                      # (784, 625)
    A = ab_table[:, bias_idxs]                   # (8, 625, 625)
    U = np.einsum('hnm,qm->hnq', A, M, optimize=True)  # (8, 625, 784)
    wins = _tap_windows()
    u_pack = np.zeros((WMAX, H * LT * L), np.float32)
    mtw = np.zeros((WMAX, L), np.float32)
    for kt, (lo, hi) in enumerate(wins):
        w = hi - lo
        for h in range(H):
            u_pack[:w, (h * LT + kt) * L:(h * LT + kt + 1) * L] = U[h, lo:hi, :]
        mtw[:w, kt * LTS:(kt + 1) * LTS] = M[kt * LTS:(kt + 1) * LTS, lo:hi].T
    u8 = np.ascontiguousarray(u_pack * USCALE).astype(FP8_NP)
    mtw8 = np.ascontiguousarray(mtw / USCALE).astype(FP8_NP)

    in_maps = []
    for c in range(N_CORES):
        xT = np.ascontiguousarray(
            x[c * BC:(c + 1) * BC].transpose(0, 2, 1)).astype(BF16_NP)
        in_maps.append({
            "xT": xT,
            "wqkvT": wqkvT,
            "wprojT": wprojT,
            "bproj": bproj2,
            "mtw8": mtw8,
            "u8": u8,
        })
    return in_maps


def _run(inputs, trace=False, **kw):
    nc = build()
    in_maps = _prep_in_maps(inputs)
    last_err = None
    for attempt in range(3):
        try:
            res = run_bass_kernel_spmd(nc, in_maps, core_ids=list(range(N_CORES)),
                                       trace=trace, **kw)
            break
        except Exception as e:  # transient NRT device errors: retry
            last_err = e
    else:
        raise last_err
    out = np.concatenate(
        [np.asarray(res.results[c]["out"], dtype=np.float32)
         for c in range(N_CORES)], axis=0)
    return out, res


def kernel(**inputs) -> np.ndarray:
    out, _ = _run(inputs, trace=False)
    return out


# revision 45
# speedup vs baseline: 1.1543x; 1.0135x over previous
"""Trainium2 Bass kernel for nn_Attention_31705448579931.

Multi-head attention (b=16, L=784, dim=384, H=8, qk=32, v=128) with a
bicubic-resampled relative-position bias:

    out = proj( softmax(q k^T/sqrt(d) + M ab M^T) v )

Sharding: data-parallel over batch - each of the 8 NeuronCores handles 2
batches and all 8 heads.

The bias B = M A M^T depends only on weights, so the host precomputes
U = A M^T per head (A symmetric) and the device adds the bias to each
S^T tile inside the PSUM accumulation group:

    S'^T[k,q] = sum_n M[k,n] U[n,q]  +  sum_d kvec[d,k] qvec[d,q]

The first sum only needs the <=93 M-rows in the 4-tap bicubic window of
each 112-row k-tile, so it is a K=93 matmul whose operands (M windows and
U, both weights-only) live in SBUF for the whole kernel in fp8_e4m3
(bias enters through exp, so ~6% fp8 quantization error on U is ~1e-3
absolute on B - negligible). This removes the bias AllGather of the
earlier design (~140us collective + ~50us PE idle) and all per-instance
bias DMA traffic, and kills the per-tile exp(B) multiply on VectorE.

Device layout highlights:
  - attention is computed k-major: S'^T tiles (kpos on partitions, q on
    free) so exp(S'^T) is directly the lhsT-side operand for the P@V
    matmul; the two N-chunks of each S^T tile run as row-group-packed
    concurrent K=32 matmuls (duplicated q/k rows at partition 32).
  - softmax denominators via an all-ones (112,128) matmul chain on the
    TensorEngine whose output replicates the column sums across
    partitions, followed by a lane-parallel fast reciprocal on VectorE
    and a column-scale fused into the PSUM->SBUF eviction of the
    attention output.
  - PV/ones accumulation chains are flushed one instance behind the
    S'^T/exp stream.
  - q/k projections run just-in-time (one 2-head group ahead); the
    partition-32 row duplication for packing is done by VectorE copies
    (no DMA packets).
  - output is stored bf16 and upcast on the host.

All attention matmuls run in bf16 with fp32 PSUM accumulation; the bias
matmul runs in fp8_e4m3 (U pre-scaled by 8, M windows by 1/8).
"""

import numpy as np
import ml_dtypes

import concourse.mybir as mybir
import concourse.tile as tile
from concourse import bacc
from concourse.bass_utils import run_bass_kernel_spmd

N_CORES = 8
B = 16          # global batch
BC = B // N_CORES  # batches per core
L = 784
DIM = 384
H = 8
QK = 32
VD = 128
RES = 25
N = RES * RES   # 625
A_CUBIC = -0.75
SCALE = QK ** -0.5

LT = 7          # l tiles of 112
LTS = 112
WMAX = 93       # max 4-tap window rows per l-tile (bias contraction)
USCALE = 8.0    # fp8 pre-scale: u *= USCALE, mtw /= USCALE
F32 = mybir.dt.float32
BF16 = mybir.dt.bfloat16
FP8 = mybir.dt.float8e4
BF16_NP = ml_dtypes.bfloat16
FP8_NP = ml_dtypes.float8_e4m3fn

NSPLITS = [(0, 512), (512, L)]  # free-dim chunks for 784-wide matmul outputs


def _cubic_weight(x):
    ax = np.abs(x)
    a = A_CUBIC
    w1 = ((a + 2.0) * ax - (a + 3.0)) * ax * ax + 1.0
    w2 = a * (((ax - 5.0) * ax + 8.0) * ax - 4.0)
    return np.where(ax <= 1.0, w1, np.where(ax < 2.0, w2, 0.0)).astype(np.float32)


def interp_matrix(Lo, Li):
    """Dense 1-D bicubic resampling matrix (Lo, Li), matches reference."""
    scale = Li / Lo
    src = (np.arange(Lo, dtype=np.float32) + 0.5) * scale - 0.5
    f = np.floor(src)
    t = (src - f).astype(np.float32)
    ws = np.stack(
        [_cubic_weight(t + 1.0), _cubic_weight(t), _cubic_weight(1.0 - t),
         _cubic_weight(2.0 - t)], axis=1)
    idx = f.astype(np.int32)[:, None] + np.arange(-1, 3, dtype=np.int32)[None, :]
    idx = np.clip(idx, 0, Li - 1)
    M = np.zeros((Lo, Li), dtype=np.float32)
    np.add.at(M, (np.arange(Lo)[:, None], idx), ws)
    return M


def _tap_windows():
    """Per l-tile row window [lo, hi) of M columns feeding that tile (4-tap)."""
    scale = N / L
    src = (np.arange(L, dtype=np.float64) + 0.5) * scale - 0.5
    f = np.floor(src).astype(np.int64)
    lo_tap = np.clip(f - 1, 0, N - 1)
    hi_tap = np.clip(f + 2, 0, N - 1)
    wins = []
    for pt in range(LT):
        sl = slice(pt * LTS, (pt + 1) * LTS)
        wins.append((int(lo_tap[sl].min()), int(hi_tap[sl].max()) + 1))
    return wins


_BUILD_CACHE = {}


def build():
    if "nc" in _BUILD_CACHE:
        return _BUILD_CACHE["nc"]

    nc = bacc.Bacc("TRN2", target_bir_lowering=False, debug=False,
                   num_devices=N_CORES)

    xT_e = nc.dram_tensor("xT", [BC, DIM, L], BF16, kind="ExternalInput")
    wqkvT_e = nc.dram_tensor("wqkvT", [DIM, 1536], BF16, kind="ExternalInput")
    wprojT_e = nc.dram_tensor("wprojT", [H * VD, DIM], BF16, kind="ExternalInput")
    bproj_e = nc.dram_tensor("bproj", [1, DIM], F32, kind="ExternalInput")
    # bias operands (weights-only, fp8): mtw8[r, kt*112+c] = M[kt*112+c,
    # lo_kt+r]/USCALE, u8[r, (h*7+kt)*784+q] = USCALE*U_h[lo_kt+r, q],
    # both 0-padded to 93 rows.
    mtw_e = nc.dram_tensor("mtw8", [WMAX, L], FP8, kind="ExternalInput")
    u_e = nc.dram_tensor("u8", [WMAX, H * LT * L], FP8, kind="ExternalInput")
    out_e = nc.dram_tensor("out", [BC, L, DIM], BF16, kind="ExternalOutput")

    with tile.TileContext(nc) as tc:
        with (
            tc.tile_pool(name="const", bufs=1) as constp,
            tc.tile_pool(name="wq", bufs=1) as wqp,
            tc.tile_pool(name="x", bufs=1) as xp,
            tc.tile_pool(name="v", bufs=1) as vp,
            tc.tile_pool(name="qk", bufs=1) as qkp,
            tc.tile_pool(name="attn", bufs=3) as attnp,
            tc.tile_pool(name="mis", bufs=1) as misp,
            tc.tile_pool(name="ot", bufs=1) as otp,
            tc.tile_pool(name="ps", bufs=4, space="PSUM") as psp,
        ):
            # ---- inputs ---------------------------------------------------
            # x first (gates V/qk projections), weights next, then the
            # resident fp8 bias operands spread across the three queues.
            xT_sb = [[None] * 3 for _ in range(BC)]
            for b in range(BC):
                for kc in range(3):
                    t = xp.tile([128, L], BF16, tag=f"x{b}{kc}")
                    (nc.sync if b == 0 else nc.gpsimd).dma_start(
                        t[:], xT_e[b, kc * 128:(kc + 1) * 128, :])
                    xT_sb[b][kc] = t
            wq_sb = []
            for kc in range(3):
                t = wqp.tile([128, 1536], BF16, tag=f"wq{kc}")
                nc.scalar.dma_start(t[:], wqkvT_e[kc * 128:(kc + 1) * 128, :])
                wq_sb.append(t)

            mtw_sb = constp.tile([WMAX, L], FP8, tag="mtw")
            nc.scalar.dma_start(mtw_sb[:], mtw_e[:, :])
            u_sb = constp.tile([WMAX, H * LT * L], FP8, tag="u8")
            for h in range(H):
                eng = (nc.gpsimd, nc.sync, nc.scalar)[h % 3]
                eng.dma_start(u_sb[:, h * LT * L:(h + 1) * LT * L],
                              u_e[:, h * LT * L:(h + 1) * LT * L])

            bproj_row = constp.tile([1, DIM], F32, tag="bprow")
            nc.gpsimd.dma_start(bproj_row[:], bproj_e[:, :])
            bpb = constp.tile([128, DIM], F32, tag="bpb")
            nc.gpsimd.partition_broadcast(bpb[:], bproj_row[:])
            ones_l = constp.tile([LTS, VD], BF16, tag="ones")
            nc.any.memset(ones_l[:], 1.0)

            wproj_sb = []
            for h in range(H):
                t = wqp.tile([128, DIM], BF16, tag=f"wp{h}")
                nc.gpsimd.dma_start(t[:], wprojT_e[h * VD:(h + 1) * VD, :])
                wproj_sb.append(t)

            # ---- PE warmup: dummy matmuls on constants run during the
            # input-DMA wait and release the HAM clock throttle
            # (1.2 -> 2.4 GHz) before the first real matmul burst.
            warm_rhs = constp.tile([LTS, 512], BF16, tag="warmr")
            nc.any.memset(warm_rhs[:], 0.5)
            warm_ps = psp.tile([VD, 512], F32, tag="ps")
            for wi in range(24):
                nc.tensor.matmul(
                    warm_ps[:],
                    lhsT=ones_l[:],
                    rhs=warm_rhs[:],
                    start=(wi == 0), stop=(wi == 23),
                )

            # ---- V projection (l-major: kpos on partitions) ---------------
            v_sb = [[None] * LT for _ in range(BC)]

            def _vproj(b):
                for lt in range(LT):
                    pv = psp.tile([LTS, 1024], F32, tag="ps")
                    for half in range(2):
                        for kc in range(3):
                            nc.tensor.matmul(
                                pv[:, half * 512:(half + 1) * 512],
                                lhsT=xT_sb[b][kc][:, lt * LTS:(lt + 1) * LTS],
                                rhs=wq_sb[kc][:, 512 + half * 512:1024 + half * 512],
                                start=(kc == 0), stop=(kc == 2),
                            )
                    vt = vp.tile([LTS, 1024], BF16, tag=f"v{b}{lt}")
                    if lt % 2 == 0:
                        nc.scalar.copy(vt[:], pv[:])
                    else:
                        nc.vector.tensor_copy(vt[:], pv[:])
                    v_sb[b][lt] = vt

            _vproj(0)

            # ---- per-head loop --------------------------------------------
            qk_sb = {}
            ot_sb = [[None] * H for _ in range(BC)]
            _pend = []

            def _qkproj(mt_i):
                """q/k projections for heads (2*mt_i, 2*mt_i+1), both
                batches, with the rows duplicated to partitions 32:64 by
                VectorE for 2-way row-group packing of the S matmuls."""
                for b in range(BC):
                    pqk = psp.tile([128, L], F32, tag="ps")
                    for (ns, ne) in NSPLITS:
                        for kc in range(3):
                            nc.tensor.matmul(
                                pqk[:, ns:ne],
                                lhsT=wq_sb[kc][:, mt_i * 128:(mt_i + 1) * 128],
                                rhs=xT_sb[b][kc][:, ns:ne],
                                start=(kc == 0), stop=(kc == 2),
                            )
                    for hh in range(2):
                        h_ = 2 * mt_i + hh
                        inst = 2 * h_ + b
                        qt = qkp.tile([2 * QK, L], BF16, tag=f"q{inst % 4}")
                        kt_ = qkp.tile([2 * QK, L], BF16, tag=f"k{inst % 4}")
                        nc.scalar.copy(qt[0:QK, :], pqk[hh * 64:hh * 64 + 32, :])
                        nc.vector.tensor_copy(kt_[0:QK, :], pqk[hh * 64 + 32:hh * 64 + 64, :])
                        nc.vector.tensor_copy(qt[QK:2 * QK, :], qt[0:QK, :])
                        nc.vector.tensor_copy(kt_[QK:2 * QK, :], kt_[0:QK, :])
                        qk_sb[(b, h_)] = (qt, kt_)

            def _flush_pv_gen(item):
                """Generator: emits the ones/PV chain matmuls in small
                steps so the caller can interleave them between the next
                instance's S' tiles - the chain matmuls keep the PE busy
                while ScalarE works through the exps, instead of the two
                phases alternating with ScalarE idle half the time."""
                fh, fb, ptiles = item
                ps_o = psp.tile([VD, L], F32, tag="ps")
                ps_one = psp.tile([VD, L], F32, tag="ps")
                steps = []
                for (ns, ne) in NSPLITS:
                    for kt in range(LT):
                        steps.append(("one", kt, ns, ne))
                for (ns, ne) in NSPLITS:
                    for kt in range(LT):
                        steps.append(("pv", kt, ns, ne))
                rdb = None
                for si, (kind, kt, ns, ne) in enumerate(steps):
                    if kind == "one":
                        nc.tensor.matmul(
                            ps_one[:, ns:ne],
                            lhsT=ones_l[:],
                            rhs=ptiles[kt][:, ns:ne],
                            start=(kt == 0), stop=(kt == LT - 1),
                        )
                        if si == LT * 2 - 1:
                            rdb = misp.tile([VD, L], F32, tag="rdb")
                            nc.vector.reciprocal_approx_fast(rdb[:], ps_one[:])
                    else:
                        nc.tensor.matmul(
                            ps_o[:, ns:ne],
                            lhsT=v_sb[fb][kt][:, fh * VD:(fh + 1) * VD],
                            rhs=ptiles[kt][:, ns:ne],
                            start=(kt == 0), stop=(kt == LT - 1),
                        )
                    yield
                ot = otp.tile([VD, L], BF16, tag=f"ot{fb}{fh}")
                nc.vector.tensor_mul(ot[:], ps_o[:], rdb[:])
                ot_sb[fb][fh] = ot
                while True:
                    yield

            def _flush_pv(item):
                g = _flush_pv_gen(item)
                for _ in range(LT * 4 + 1):
                    next(g)

            def _proj(b):
                for lt in range(LT):
                    py = psp.tile([LTS, DIM], F32, tag="ps")
                    for h in range(H):
                        nc.tensor.matmul(
                            py[:],
                            lhsT=ot_sb[b][h][:, lt * LTS:(lt + 1) * LTS],
                            rhs=wproj_sb[h][:],
                            start=(h == 0), stop=(h == H - 1),
                        )
                    # distinct ysb buffer per tile: with a single shared
                    # buffer each add had to wait for the previous store
                    # DMA to finish, serializing the whole drain at
                    # ~2.9us/tile and blocking the last flush's DVE ops.
                    ysb = misp.tile([LTS, DIM], BF16, tag=f"y{b}{lt}")
                    nc.vector.tensor_add(ysb[:], py[:], bpb[:LTS, :])
                    eng = (nc.sync, nc.scalar, nc.gpsimd)[lt % 3]
                    eng.dma_start(out_e[b, lt * LTS:(lt + 1) * LTS, :], ysb[:])

            _qkproj(0)
            _vproj(1)
            for h in range(H):
                for b in range(BC):
                    # prefetch the next 2-head group's q/k projections one
                    # head ahead of first use so the CASTs overlap the
                    # current instances instead of stalling the next ones.
                    if len(_pend) > 1:
                        _flush_pv(_pend.pop(0))
                    # prefetch after the flush: the chain matmuls keep the
                    # PE busy while exps drain the psum pool, so the pqk
                    # allocation no longer stalls the engine (observed
                    # 4.4us gap -> HAM re-throttle cascade when emitted
                    # before the flush).
                    if h % 2 == 1 and b == 0 and h < H - 1:
                        _qkproj((h + 1) // 2)
                    qt, kt_ = qk_sb[(b, h)]
                    pt_tiles = []
                    for kt in range(LT):
                        ps_s = psp.tile([LTS, L], F32, tag="ps")
                        # bias: B^T tile = mtw8^T @ u8 window (fp8, K=93)
                        for (ns, ne) in NSPLITS:
                            nc.tensor.matmul(
                                ps_s[:, ns:ne],
                                lhsT=mtw_sb[:, kt * LTS:(kt + 1) * LTS],
                                rhs=u_sb[:, (h * LT + kt) * L + ns:(h * LT + kt) * L + ne],
                                start=True, stop=False,
                            )
                        # S: the two N-chunks run as row-group-packed
                        # concurrent K=32 matmuls accumulating onto B^T.
                        for wi, (ns, ne) in enumerate(NSPLITS):
                            o0 = wi * QK
                            nc.tensor.matmul(
                                ps_s[:, ns:ne],
                                lhsT=kt_[o0:o0 + QK, kt * LTS:(kt + 1) * LTS],
                                rhs=qt[o0:o0 + QK, ns:ne],
                                start=False, stop=True,
                            )
                        ptile = attnp.tile([LTS, L], BF16, tag=f"pT{kt}")
                        nc.scalar.activation(ptile[:], ps_s[:],
                                             mybir.ActivationFunctionType.Exp)
                        pt_tiles.append(ptile)
                    _pend.append((h, b, pt_tiles))

            # ---- drain: flush both remaining instances first (their
            # chains have no unmet deps), then both projections - the
            # projections' ot operands are DVE-produced, so putting the
            # chains first keeps the PE busy while DVE catches up.
            while _pend:
                _flush_pv(_pend.pop(0))
            for b in range(BC):
                _proj(b)

    nc.compile()
    _BUILD_CACHE["nc"] = nc
    return nc


def _prep_in_maps(inputs):
    x = np.asarray(inputs["x"], dtype=np.float32)
    Wqkv = np.asarray(inputs["Wqkv"], dtype=np.float32)
    Wproj = np.asarray(inputs["Wproj"], dtype=np.float32)
    bproj = np.asarray(inputs["bproj"], dtype=np.float32)
    ab_table = np.asarray(inputs["ab_table"], dtype=np.float32)
    bias_idxs = np.asarray(inputs["bias_idxs"])

    # reorder qkv weight rows: [q0 k0 q1 k1 ... q7 k7 | v0 v1 ... v7]
    w3 = Wqkv.reshape(H, 2 * QK + VD, DIM)
    qk_rows = np.concatenate(
        [np.concatenate([w3[h, :QK] * SCALE, w3[h, QK:2 * QK]], axis=0)
         for h in range(H)],
        axis=0)                     # (512, 384)
    v_rows = np.concatenate([w3[h, 2 * QK:] for h in range(H)], axis=0)  # (1024,384)
    wqkvT = np.ascontiguousarray(
        np.concatenate([qk_rows, v_rows], axis=0).T).astype(BF16_NP)  # (384,1536)

    wprojT = np.ascontiguousarray(Wproj.T).astype(BF16_NP)  # (1024, 384)
    bproj2 = np.ascontiguousarray(bproj.reshape(1, DIM))

    # bias fold: B_h = M A_h M^T with A_h symmetric; device computes
    # B^T[k,q] = sum_{n in win} M[k,n] U_h[n,q] as a K=93 fp8 matmul.
    M = interp_matrix(L, N)                      # (784, 625)
    A = ab_table[:, bias_idxs]                   # (8, 625, 625)
    U = np.einsum('hnm,qm->hnq', A, M, optimize=True)  # (8, 625, 784)
    wins = _tap_windows()
    u_pack = np.zeros((WMAX, H * LT * L), np.float32)
    mtw = np.zeros((WMAX, L), np.float32)
    for kt, (lo, hi) in enumerate(wins):
        w = hi - lo
        for h in range(H):
            u_pack[:w, (h * LT + kt) * L:(h * LT + kt + 1) * L] = U[h, lo:hi, :]
        mtw[:w, kt * LTS:(kt + 1) * LTS] = M[kt * LTS:(kt + 1) * LTS, lo:hi].T
    u8 = np.ascontiguousarray(u_pack * USCALE).astype(FP8_NP)
    mtw8 = np.ascontiguousarray(mtw / USCALE).astype(FP8_NP)

    in_maps = []
    for c in range(N_CORES):
        xT = np.ascontiguousarray(
            x[c * BC:(c + 1) * BC].transpose(0, 2, 1)).astype(BF16_NP)
        in_maps.append({
            "xT": xT,
            "wqkvT": wqkvT,
            "wprojT": wprojT,
            "bproj": bproj2,
            "mtw8": mtw8,
            "u8": u8,
        })
    return in_maps


def _run(inputs, trace=False, **kw):
    nc = build()
    in_maps = _prep_in_maps(inputs)
    last_err = None
    for attempt in range(3):
        try:
            res = run_bass_kernel_spmd(nc, in_maps, core_ids=list(range(N_CORES)),
                                       trace=trace, **kw)
            break
        except Exception as e:  # transient NRT device errors: retry
            last_err = e
    else:
        raise last_err
    out = np.concatenate(
        [np.asarray(res.results[c]["out"], dtype=np.float32)
         for c in range(N_CORES)], axis=0)
    return out, res


def kernel(**inputs) -> np.ndarray:
    out, _ = _run(inputs, trace=False)
    return out
